# revision 19
# baseline (speedup 1.0000x reference)
"""GCN layer (SpMM + Linear + LayerNorm + ReLU) on 8 Trainium2 NeuronCores.

Strategy (node sharding):
  - Host sorts edges by destination row; core c owns rows [c*RPC, (c+1)*RPC).
  - Rows are processed in PSUM windows of 512; each window splits into
    128-row blocks; edges are grouped per (window, col-region, block) cell
    and packed into 128-edge chunks (slots). The schedule (slot counts) is
    the max over cores so one SPMD program serves all 8 cores.
  - Gather: nc.gpsimd.dma_gather with int16 indices (4 col-regions of 25000
    rows each). x is stored as sliding pairs x_dup[i] = [x[i] | x[i+1]] in
    bf16 so each 256B element (the dma_gather minimum) carries the needed
    row in its first half.
  - Aggregation: TensorE computes aggT[64f, rows] += Xg[128e, :64].T @ S
    where S is a host-built one-hot-times-val matrix (bf16, 128 row cols).
  - Linear+LayerNorm fused: centering folded into weights (WTc, bc), bias
    via a ones-row; var from Square-activation accumulate; out = relu(v*rstd)
    on the gamma=1/beta=0 fast path (general path uses vector ops).
"""

import numpy as np
import ml_dtypes

N_NODES = 100000
DIM = 64
LN_EPS = 1e-5
NCORES = 8

WIN = 512        # rows per PSUM window
SUB = 128        # rows per block (static matmul base + S width)
PCHUNK = 128     # edges per chunk
REGION = 25000   # col-region size for int16 gather indices
NREGION = 4
CALL_SLOTS = 8   # max chunks per dma_gather call (1024 indices)


def _host_prep(edge_row, edge_col, edge_val, n_nodes, ncores):
    rpc = n_nodes // ncores
    nwin = (rpc + WIN - 1) // WIN
    nregion = (n_nodes + REGION - 1) // REGION

    er = np.asarray(edge_row).astype(np.int64)
    ec = np.asarray(edge_col).astype(np.int64)
    ev = np.asarray(edge_val).astype(np.float32)

    core = er // rpc
    lr = er - core * rpc
    w = lr // WIN
    b = (lr % WIN) // SUB
    q = ec // REGION
    blk_per_win = [min(WIN, rpc - wi * WIN + 0) for wi in range(nwin)]
    nblk = [(r + SUB - 1) // SUB for r in blk_per_win]
    # global cell index: (w, q, b) -> contiguous, per-window-variable block count
    cell_off_w = np.concatenate([[0], np.cumsum([nregion * nb for nb in nblk])])
    ncells = int(cell_off_w[-1])
    cell = cell_off_w[w] + q * np.asarray(nblk)[w] + b

    # order edges by (core, cell, row)
    order = np.lexsort((lr, cell, core))
    core_s = core[order]
    cell_s = cell[order]
    lr_s = lr[order]
    ec_s = ec[order]
    ev_s = ev[order]

    flat = core_s * ncells + cell_s
    bc = np.bincount(flat, minlength=ncores * ncells)
    counts = bc.reshape(ncores, ncells)

    slots = -(-counts.max(axis=0) // PCHUNK)  # ceil
    slot_off = np.concatenate([[0], np.cumsum(slots)]).astype(np.int64)
    tot = max(int(slot_off[-1]), 1)

    # per-core arrays
    idx_flat = np.zeros((ncores, tot * PCHUNK), np.int16)
    sv = np.zeros((ncores, 128, tot * SUB), ml_dtypes.float8_e4m3)
    val = np.zeros((ncores, 128, tot), np.float32)

    # vectorized fill
    # edge position within its (core, cell) group:
    grp_start = np.zeros(ncores * ncells + 1, np.int64)
    np.cumsum(bc, out=grp_start[1:])
    j = np.arange(len(order), dtype=np.int64) - grp_start[flat]
    slot = slot_off[cell_s] + j // PCHUNK
    p = j % PCHUNK
    # block row base of each edge's cell
    # recover (w, b) per edge from sorted arrays:
    w_s = lr_s // WIN
    b_s = (lr_s % WIN) // SUB
    base_s = w_s * WIN + b_s * SUB
    rl = lr_s - base_s
    idx_local = (ec_s % REGION).astype(np.int16)
    for c in range(ncores):
        m = core_s == c
        idx_flat[c][slot[m] * PCHUNK + p[m]] = idx_local[m]
        sv[c][p[m], slot[m] * SUB + rl[m]] = 1.0
        val[c][p[m], slot[m]] = ev_s[m]

    # wrapped idx layout [128, tot*8] (16-partition wrap, replicated x8)
    idx16 = np.zeros((ncores, 128, tot * 8), np.int16)
    v = idx_flat.reshape(ncores, tot * 8, 16)
    wrap = np.swapaxes(v, 1, 2)  # [ncores, 16, tot*8]
    idx16[:] = np.tile(wrap, (1, 8, 1))

    # schedule: per window -> per (q, b) cells and gather calls
    sched_win = []
    for wi in range(nwin):
        wrows = min(WIN, rpc - wi * WIN)
        nb = nblk[wi]
        cells = []
        for qi in range(nregion):
            for bi in range(nb):
                ci = int(cell_off_w[wi] + qi * nb + bi)
                cells.append((qi, bi, int(slot_off[ci]), int(slots[ci])))
        s0 = int(slot_off[cell_off_w[wi]])
        s1 = int(slot_off[cell_off_w[wi + 1]]) if wi + 1 < len(cell_off_w) else int(
            slot_off[-1]
        )
        # gather calls: contiguous slot ranges within one region
        calls = []
        for qi in range(nregion):
            qcells = [c for c in cells if c[0] == qi and c[3] > 0]
            if not qcells:
                continue
            qs0 = min(c[2] for c in qcells)
            qs1 = max(c[2] + c[3] for c in qcells)
            s = qs0
            while s < qs1:
                ns = min(CALL_SLOTS, qs1 - s)
                calls.append((qi, s, ns))
                s += ns
        sched_win.append({
            "wrows": wrows,
            "nblk": nb,
            "s0": s0,
            "s1": s1,
            "cells": cells,
            "calls": calls,
        })

    return {
        "rpc": rpc,
        "nwin": nwin,
        "tot": tot,
        "idx16": idx16,
        "sv": sv,
        "val": val,
        "sched_win": sched_win,
    }


def _build_program(nc, sched, n_nodes, fastpath):
    from contextlib import ExitStack
    import concourse.bass as bass
    import concourse.tile as tile
    from concourse import mybir

    f32 = mybir.dt.float32
    bf16 = mybir.dt.bfloat16
    fp8 = mybir.dt.float8e4
    i16 = mybir.dt.int16
    AF = mybir.ActivationFunctionType

    rpc = sched["rpc"]
    tot = sched["tot"]
    sched_win = sched["sched_win"]

    xdup = nc.dram_tensor("xdup", [n_nodes, 2 * DIM], bf16, kind="ExternalInput")
    idxd = nc.dram_tensor("idx", [128, tot * 8], i16, kind="ExternalInput")
    svd = nc.dram_tensor("sv", [128, tot * SUB], fp8, kind="ExternalInput")
    vald = nc.dram_tensor("val", [128, tot], f32, kind="ExternalInput")
    wtbd = nc.dram_tensor("wtb", [DIM + 1, DIM], f32, kind="ExternalInput")
    gbd = nc.dram_tensor("gb", [2, DIM], f32, kind="ExternalInput")
    outd = nc.dram_tensor("out", [rpc, DIM], f32, kind="ExternalOutput")

    max_sw = max(s["s1"] - s["s0"] for s in sched_win)
    nbufs = 4

    with tile.TileContext(nc) as tc, ExitStack() as ctx:
        singles = ctx.enter_context(tc.tile_pool(name="singles", bufs=1))
        wpool = ctx.enter_context(tc.tile_pool(name="win", bufs=nbufs))
        gpool = ctx.enter_context(tc.tile_pool(name="grp", bufs=3))
        pagg = ctx.enter_context(tc.tile_pool(name="pagg", bufs=2, space="PSUM"))
        pv = ctx.enter_context(tc.tile_pool(name="pv", bufs=4, space="PSUM"))

        zeros = singles.tile([128, WIN], bf16)
        nc.vector.memset(zeros[:], 0.0)
        eps_s = singles.tile([128, 1], f32)
        nc.vector.memset(eps_s[:], LN_EPS)
        wtb_s = singles.tile([DIM + 1, DIM], f32)
        nc.sync.dma_start(out=wtb_s[:], in_=wtbd[:])
        if not fastpath:
            gam_s = singles.tile([128, DIM], f32)
            bet_s = singles.tile([128, DIM], f32)
            gsrc = gbd.ap()
            nc.sync.dma_start(
                out=gam_s[:],
                in_=bass.AP(tensor=gsrc.tensor, offset=0, ap=[[0, 128], [1, DIM]]),
            )
            nc.sync.dma_start(
                out=bet_s[:],
                in_=bass.AP(tensor=gsrc.tensor, offset=DIM, ap=[[0, 128], [1, DIM]]),
            )

        for wi, swin in enumerate(sched_win):
            wrows = swin["wrows"]
            s0, s1 = swin["s0"], swin["s1"]
            sw = s1 - s0
            if sw == 0:
                continue
            wstart = wi * WIN

            idx_t = wpool.tile([128, max_sw * 8], i16, tag="idx")
            nc.sync.dma_start(
                out=idx_t[:, :sw * 8], in_=idxd[:, s0 * 8:s1 * 8]
            )
            sv_t = wpool.tile([128, max_sw * SUB], fp8, tag="sv")
            nc.sync.dma_start(
                out=sv_t[:, :sw * SUB], in_=svd[:, s0 * SUB:s1 * SUB]
            )
            val_t = wpool.tile([128, max_sw], f32, tag="val")
            nc.sync.dma_start(out=val_t[:, :sw], in_=vald[:, s0:s1])
            xg_t = wpool.tile([128, max_sw, 2 * DIM], bf16, tag="xg")
            for ci, (qi, cs0, cns) in enumerate(swin["calls"]):
                lo = cs0 - s0
                nc.gpsimd.dma_gather(
                    out_ap=xg_t[:, lo:lo + cns, :],
                    in_ap=xdup[qi * REGION:min((qi + 1) * REGION, n_nodes), :],
                    idxs_ap=idx_t[:, lo * 8:(lo + cns) * 8],
                    num_idxs=cns * PCHUNK,
                    num_idxs_reg=cns * PCHUNK,
                    elem_size=2 * DIM,
                )
            # scale gathered rows by per-edge val (one batched pass; val is
            # broadcast over the 64 used feature columns of each slot)
            _vt = val_t[:, :sw]
            val_b = bass.AP(
                tensor=_vt.tensor,
                offset=_vt.offset,
                ap=list(_vt.ap) + [[0, DIM]],
            )
            nc.vector.tensor_mul(
                out=xg_t[:, :sw, 0:DIM],
                in0=xg_t[:, :sw, 0:DIM],
                in1=val_b,
            )

            agg_ps = pagg.tile([DIM, WIN], f32, tag="agg")
            nc.tensor.matmul(
                out=agg_ps[:, :wrows],
                lhsT=zeros[:, :DIM],
                rhs=zeros[:, :wrows],
                start=True,
                stop=False,
                skip_group_check=True,
            )
            mm_list = []
            for (qi, bi, cso, csn) in swin["cells"]:
                base = bi * SUB
                nsub = min(SUB, wrows - base)
                for s in range(cso - s0, cso - s0 + csn):
                    mm_list.append((base, nsub, s))
            for k, (base, nsub, s) in enumerate(mm_list):
                nc.tensor.matmul(
                    out=agg_ps[:, base:base + nsub],
                    lhsT=xg_t[:, s, 0:DIM],
                    rhs=sv_t[:, s * SUB:s * SUB + nsub],
                    start=False,
                    stop=k == len(mm_list) - 1,
                    skip_group_check=True,
                )

            aggb = wpool.tile([DIM + 1, WIN], f32, tag="aggb")
            nc.scalar.copy(out=aggb[0:DIM, :wrows], in_=agg_ps[:, :wrows])
            nc.vector.memset(aggb[DIM:DIM + 1, :wrows], 1.0)

            ngrp = (wrows + 127) // 128
            ssq = gpool.tile([128, WIN // 128], f32, tag="ssq")
            rstd = gpool.tile([128, WIN // 128], f32, tag="rstd")
            o_t = gpool.tile([128, WIN // 128, DIM], f32, tag="ot")
            v_list = []
            for g in range(ngrp):
                m = min(128, wrows - g * 128)
                v_ps = pv.tile([128, DIM], f32, tag="v")
                nc.tensor.matmul(
                    out=v_ps[:m, :],
                    lhsT=aggb[:, g * 128:g * 128 + m],
                    rhs=wtb_s[:, :],
                    start=True,
                    stop=True,
                )
                sq = gpool.tile([128, DIM], f32, tag="sq")
                nc.scalar.activation(
                    out=sq[:m, :],
                    in_=v_ps[:m, :],
                    func=AF.Square,
                    accum_out=ssq[:m, g:g + 1],
                )
                v_list.append((g, m, v_ps))

            nc.scalar.activation(
                out=rstd[:, :ngrp],
                in_=ssq[:, :ngrp],
                func=AF.Sqrt,
                bias=eps_s[:, :],
                scale=1.0 / DIM,
            )
            nc.vector.reciprocal(out=rstd[:, :ngrp], in_=rstd[:, :ngrp])

            for g, m, v_ps in v_list:
                if fastpath:
                    nc.scalar.activation(
                        out=o_t[:m, g, :],
                        in_=v_ps[:m, :],
                        func=AF.Relu,
                        scale=rstd[:m, g:g + 1],
                    )
                else:
                    nc.scalar.mul(
                        out=o_t[:m, g, :], in_=v_ps[:m, :], mul=rstd[:m, g:g + 1]
                    )
                    nc.vector.tensor_mul(
                        out=o_t[:m, g, :], in0=o_t[:m, g, :], in1=gam_s[:m, :]
                    )
                    nc.vector.tensor_add(
                        out=o_t[:m, g, :], in0=o_t[:m, g, :], in1=bet_s[:m, :]
                    )
                    nc.vector.tensor_scalar_max(
                        out=o_t[:m, g, :], in0=o_t[:m, g, :], scalar1=0.0
                    )

            if wrows % 128 == 0:
                dst = outd[wstart:wstart + wrows, :].rearrange(
                    "(g p) f -> p g f", p=128
                )
                nc.sync.dma_start(out=dst, in_=o_t[:, :ngrp, :])
            else:
                for g in range(ngrp):
                    m = min(128, wrows - g * 128)
                    nc.sync.dma_start(
                        out=outd[wstart + g * 128:wstart + g * 128 + m, :],
                        in_=o_t[:m, g, :],
                    )


def _execute(inputs, n_nodes=N_NODES, ncores=NCORES, trace=False, trace_cores=None):
    from concourse import bacc
    from concourse.bass_utils import run_bass_kernel_spmd

    x = np.asarray(inputs["x"], np.float32)
    W = np.asarray(inputs["W"], np.float32)
    b = np.asarray(inputs["b"], np.float32)
    gamma = np.asarray(inputs["gamma"], np.float32)
    beta = np.asarray(inputs["beta"], np.float32)

    sched = _host_prep(
        inputs["edge_row"], inputs["edge_col"], inputs["edge_val"], n_nodes, ncores
    )

    xdup = np.zeros((n_nodes, 2 * DIM), np.float32)
    xdup[:, :DIM] = x
    xdup[:-1, DIM:] = x[1:]
    xdup = xdup.astype(ml_dtypes.bfloat16)

    WT = W.T.astype(np.float32)
    WTc = WT - WT.mean(axis=1, keepdims=True)
    bc = (b - b.mean()).astype(np.float32)
    wtb = np.concatenate([WTc, bc[None, :]], axis=0).astype(np.float32)
    gb = np.stack([gamma, beta], axis=0).astype(np.float32)

    fastpath = bool(np.all(gamma == 1.0) and np.all(beta == 0.0))

    nc = bacc.Bacc(
        "TRN2", target_bir_lowering=False, debug=False, num_devices=ncores
    )
    _build_program(nc, sched, n_nodes, fastpath)
    nc.compile()

    in_maps = [
        {
            "xdup": xdup,
            "idx": np.ascontiguousarray(sched["idx16"][c]),
            "sv": np.ascontiguousarray(sched["sv"][c]),
            "val": np.ascontiguousarray(sched["val"][c]),
            "wtb": wtb,
            "gb": gb,
        }
        for c in range(ncores)
    ]
    r = run_bass_kernel_spmd(
        nc,
        in_maps,
        list(range(ncores)),
        trace=trace,
        trace_cores=trace_cores,
    )
    out = np.concatenate([r.results[c]["out"] for c in range(ncores)], axis=0)
    return out.astype(np.float32), r


def kernel(**inputs):
    out, _ = _execute(inputs)
    return out
